# revision 47
# baseline (speedup 1.0000x reference)
"""ChebyKAN layer (degree-7) on 8 Trainium2 NeuronCores.

out[b,o] = sum_{i,d} T_d(tanh(x[b,i])) * C[o,i,d]  +  x @ BW.T

V3 strategy (evolution of V2):
  - Data-parallel over batch: 16384 rows -> 8 cores x 2048.
  - T_0 == 1 folded into a host-precomputed bias[o].
  - Cheby matmuls in fp8e4m3 DoubleRow (K=256/instr, N=512); coeffs
    host-prescaled by 2**16 with a per-degree scale absorbing the
    basis-tile normalization. Base matmul in bf16 (x and base_weight
    both bf16, prescaled by 2**16) accumulating into the same PSUM;
    eviction rescales by 2**-16, adds bias, writes bf16.
  - Basis tiles: U1 = tanh(x) (fp8 cast of the single tanh), then a
    product/square ladder on DVE (all bf16, tensor_tensor 2x /
    tensor_scalar 4x):
        V2 = (-2t)(2t) + 2          = -2*T2
        V3 = (2t)V2 + 2t            = -2*T3
        V4 = V2^2 - 2               = +2*T4
        V5 = V2*V3 + (-2t)          = +2*T5
        V6 = -(V3^2) + 2            = -2*T6
        V7 = V3*V4 + 2t             = -2*T7
    Casts to fp8 are pair-fused on ACT ([U2,U3], [U4,U5], [U6,U7])
    into a per-pair mega-tile [128, 7, 2, F].
  - PSUM as 8 single-bank [128,512] tiles (o4 x h); each evicted the
    moment its accumulation group stops, alternating ACT/DVE, with
    the output DMA issued per 512-col chunk (bf16).
  - ~26 dummy warmup matmuls at program start (on memset scratch)
    warm the PE HAM clock-gate during the initial DMA window.
  - out_features live on PSUM partitions: x ships pre-transposed
    (xT, bf16) and outT (bf16) is transposed/upcast on the host.
"""

import numpy as np

import concourse.mybir as mybir
from concourse import bacc, tile
from concourse.bass_utils import run_bass_kernel_spmd

IN_F = 1024
OUT_F = 1024
DEG = 7
N_CORES = 8
SC = float(2 ** 16)
# basis tile d holds GAMMA[d-1] * T_d(tanh x)
GAMMA = np.array([1.0, -2.0, -2.0, 2.0, 2.0, -2.0, -2.0])

F32 = mybir.dt.float32
BF16 = mybir.dt.bfloat16
FP8 = mybir.dt.float8e4
ALU = mybir.AluOpType
ACTF = mybir.ActivationFunctionType
DR = mybir.MatmulPerfMode.DoubleRow


def _build_program(b_core: int, n_cores: int = N_CORES):
    bsup = min(1024, b_core)
    assert b_core % bsup == 0
    n_bs = b_core // bsup
    F = bsup
    n_ci = IN_F // 128            # 8
    n_pair = n_ci // 2            # 4
    n_oh = 2

    nc = bacc.Bacc("TRN2", target_bir_lowering=False, debug=False,
                   num_devices=n_cores)
    xT = nc.dram_tensor("xT", [IN_F, b_core], BF16, kind="ExternalInput")
    w8 = nc.dram_tensor("w8", [n_oh, n_pair, 128, DEG * 2 * 512], FP8,
                        kind="ExternalInput")
    wb = nc.dram_tensor("wb", [n_pair, 128, 2048], BF16,
                        kind="ExternalInput")
    biasm = nc.dram_tensor("biasm", [128, 8], F32, kind="ExternalInput")
    outT = nc.dram_tensor("outT", [OUT_F, b_core], BF16,
                          kind="ExternalOutput")

    with tile.TileContext(nc) as tc:
        with (
            tc.tile_pool(name="const", bufs=1) as cpool,
            tc.tile_pool(name="xp", bufs=4) as xpool,
            tc.tile_pool(name="head", bufs=4) as hpool,
            tc.tile_pool(name="mt", bufs=3) as mpool,
            tc.tile_pool(name="vb", bufs=4) as vpool,
            tc.tile_pool(name="t8", bufs=5) as t8pool,
            tc.tile_pool(name="w8p", bufs=3) as wpool,
            tc.tile_pool(name="wbp", bufs=4) as wbpool,
            tc.tile_pool(name="op", bufs=3) as opool,
            tc.tile_pool(name="ps", bufs=8, space="PSUM") as ppool,
        ):
            # ---- warmup: dummy matmuls spin the PE so the HAM
            # clock-gate reaches K=8/8 while the first input DMAs are
            # in flight ----
            warm_w = cpool.tile([128, 128], BF16)
            warm_x = cpool.tile([128, 256], BF16)
            nc.gpsimd.memset(warm_w[:], 0.0)
            nc.gpsimd.memset(warm_x[:], 0.0)
            warm_ps = ppool.tile([128, 512], F32, tag="ps",
                                 name="warm_ps")
            for _ in range(16):
                nc.tensor.matmul(warm_ps[:, 0:256], warm_w[:],
                                 warm_x[:], start=True, stop=True)
            bias_sb = cpool.tile([128, 8], F32)

            # ---- persistent base-weight tiles (one per bpair, both
            # oh halves; 4 KiB DMA lines) ----
            wbt = {}

            def load_wb(bp, split=False):
                t = wbpool.tile([128, 2048], BF16, tag="wb",
                                name=f"wb_{bp}")
                if split:
                    # latency-critical: halve transfer time by running
                    # the partition halves on both hwdge queues
                    nc.sync.dma_start(t[0:64, :], wb[bp, 0:64, :])
                    nc.scalar.dma_start(t[64:128, :], wb[bp, 64:128, :])
                else:
                    nc.sync.dma_start(t[:], wb[bp, :, :])
                wbt[bp] = t

            load_wb(0, split=True)

            # w8 prefetch cache, keyed by the consuming (bs, oh, pair)
            # since wpool buffers rotate
            w8t = {}

            def load_w8(bs, oh, pair, split=False):
                if (bs, oh, pair) in w8t:
                    return w8t[(bs, oh, pair)]
                t = wpool.tile([128, DEG * 2 * 512], FP8, tag="w8",
                               name=f"w8_{bs}_{oh}_{pair}")
                if split:
                    # first-needed degrees land first
                    nc.sync.dma_start(t[:, 0:2 * 1024],
                                      w8[oh, pair, :, 0:2 * 1024])
                    nc.sync.dma_start(t[:, 2 * 1024:],
                                      w8[oh, pair, :, 2 * 1024:])
                else:
                    nc.sync.dma_start(t[:], w8[oh, pair, :, :])
                w8t[(bs, oh, pair)] = t
                return t

            xts = {}
            t8s = {}

            def run_basis(bs, pair, cols):
                """Basis ladder over a column slice (cols within
                [0, F)), both i-planes at once. Writes fp8 tiles into
                t8s[(bs, pair)][:, d-1, :, cols]."""
                t8p = t8s[(bs, pair)]
                xt = xts[pair]
                xcols = slice(bs * F + cols.start, bs * F + cols.stop)
                n = cols.stop - cols.start
                W = 2 * n

                def vv(t):  # [128, 2n] -> [128, 2, n]
                    return t[:].rearrange("p (two f) -> p two f", two=2)

                def vb(t):  # [128, 2, 2n] -> [128, 2, 2, n]
                    return t[:].rearrange("p v (two f) -> p v two f",
                                          two=2)

                xnb = hpool.tile([128, W], BF16, tag="h")
                nc.scalar.activation(vv(xnb), xt[:, :, xcols], ACTF.Tanh)
                # U1 = tanh(x) directly (gamma_1 = 1)
                nc.scalar.copy(t8p[:, 0, :, cols], vv(xnb))
                w1 = hpool.tile([128, W], BF16, tag="h")
                nc.vector.tensor_scalar_mul(w1[:], xnb[:], 2.0)

                vb23 = vpool.tile([128, 2, W], BF16, tag="v")
                v2, v3 = vb23[:, 0, :], vb23[:, 1, :]
                m2 = mpool.tile([128, W], BF16, tag="m")
                nc.vector.tensor_mul(m2[:], w1[:], w1[:])
                nc.vector.tensor_scalar(v2, m2[:], -1.0, 2.0,
                                        ALU.mult, ALU.add)
                m3 = mpool.tile([128, W], BF16, tag="m")
                nc.vector.tensor_mul(m3[:], w1[:], v2)
                nc.vector.tensor_add(v3, m3[:], w1[:])
                nc.scalar.copy(t8p[:, 1:3, :, cols], vb(vb23))

                vb45 = vpool.tile([128, 2, W], BF16, tag="v")
                v4, v5 = vb45[:, 0, :], vb45[:, 1, :]
                m4 = mpool.tile([128, W], BF16, tag="m")
                nc.vector.tensor_mul(m4[:], v2, v2)
                nc.vector.tensor_scalar_add(v4, m4[:], -2.0)
                m5 = mpool.tile([128, W], BF16, tag="m")
                nc.vector.tensor_mul(m5[:], w1[:], v4)
                nc.vector.tensor_add(v5, m5[:], v3)
                nc.scalar.copy(t8p[:, 3:5, :, cols], vb(vb45))

                vb67 = vpool.tile([128, 2, W], BF16, tag="v")
                v6, v7 = vb67[:, 0, :], vb67[:, 1, :]
                m6 = mpool.tile([128, W], BF16, tag="m")
                nc.vector.tensor_mul(m6[:], v3, v3)
                nc.vector.tensor_scalar(v6, m6[:], -1.0, 2.0,
                                        ALU.mult, ALU.add)
                m7 = mpool.tile([128, W], BF16, tag="m")
                nc.vector.tensor_mul(m7[:], v3, v4)
                nc.vector.tensor_add(v7, m7[:], w1[:])
                nc.scalar.copy(t8p[:, 5:7, :, cols], vb(vb67))

            def evict(bs, oh, o4, h, po_t, on_scalar, obt, fin=False):
                """Evict one [128,512] PSUM bank into its half of the
                shared [128,1024] output tile; DMA the full tile (2 KiB
                DRAM lines) once the h=1 half lands."""
                oc = oh * 4 + o4
                ob = obt[:, h * 512:(h + 1) * 512]
                bias_col = bias_sb[:, oc:oc + 1]
                if fin:
                    # final tile: half on each engine, then a
                    # partition-split DMA on both hwdge queues
                    nc.scalar.activation(ob[:, 0:256], po_t[:, 0:256],
                                         ACTF.Identity, bias=bias_col,
                                         scale=1.0 / SC)
                    nc.vector.tensor_scalar(ob[:, 256:512],
                                            po_t[:, 256:512], 1.0 / SC,
                                            bias_col, ALU.mult, ALU.add)
                elif on_scalar:
                    nc.scalar.activation(ob, po_t[:], ACTF.Identity,
                                         bias=bias_col, scale=1.0 / SC)
                else:
                    nc.vector.tensor_scalar(ob, po_t[:], 1.0 / SC,
                                            bias_col, ALU.mult, ALU.add)
                if h == 1:
                    if fin:
                        nc.scalar.dma_start(
                            outT[oc * 128:oc * 128 + 64,
                                 bs * F:(bs + 1) * F], obt[0:64, :])
                        nc.sync.dma_start(
                            outT[oc * 128 + 64:(oc + 1) * 128,
                                 bs * F:(bs + 1) * F], obt[64:128, :])
                    else:
                        nc.sync.dma_start(
                            outT[oc * 128:(oc + 1) * 128,
                                 bs * F:(bs + 1) * F], obt[:, :])

            # ---- x loaded once for the whole core (both supertiles;
            # full 4 KiB xT rows per DMA descriptor except the
            # latency-critical first-pair first-supertile planes) ----
            for pair in range(n_pair):
                xt = xpool.tile([128, 2, b_core], BF16, tag="x",
                                name=f"x_{pair}")
                if pair == 0:
                    for q in range(4):
                        eng = nc.sync if q % 2 == 0 else nc.scalar
                        eng.dma_start(xt[q * 32:(q + 1) * 32, 0, 0:F],
                                      xT[q * 32:(q + 1) * 32, 0:F])
                    nc.sync.dma_start(xt[0:64, 1, 0:F],
                                      xT[128:192, 0:F])
                    nc.scalar.dma_start(xt[64:128, 1, 0:F],
                                        xT[192:256, 0:F])
                    load_w8(0, 0, 0, split=True)
                    load_wb(1)
                    # dummy tanh pre-loads the ACT table set right
                    # after the scalar-queue DMA issues (the ~2.7us
                    # table load must not delay those)
                    warm_act = cpool.tile([128, 128], BF16)
                    nc.scalar.activation(warm_act[:], warm_w[:],
                                         ACTF.Tanh)
                else:
                    nc.sync.dma_start(
                        xt[:, :, :],
                        xT[pair * 256:(pair + 1) * 256, :].rearrange(
                            "(two k) f -> k two f", two=2))
                    if pair == 1:
                        load_w8(0, 0, 1)
                        nc.sync.dma_start(bias_sb[:], biasm[:, :])
                xts[pair] = xt
            load_wb(2)
            load_wb(3)
            if n_bs > 1:
                # first-pair second-supertile columns, needed last
                nc.sync.dma_start(
                    xts[0][:, :, F:],
                    xT[0:256, F:].rearrange("(two k) f -> k two f",
                                            two=2))

            for bs in range(n_bs):

                # basis production (column-split for the first pairs
                # of the first supertile: they are latency-critical)
                for pair in range(n_pair):
                    t8s[(bs, pair)] = t8pool.tile(
                        [128, DEG, 2, F], FP8, tag="t8",
                        name=f"t8_{bs}_{pair}")
                    if bs == 0:
                        run_basis(bs, pair, slice(0, F // 2))
                        run_basis(bs, pair, slice(F // 2, F))
                    else:
                        run_basis(bs, pair, slice(0, F))

                # matmul passes
                for oh in range(n_oh):
                    first_pass = (bs == 0 and oh == 0)
                    base_before = {p: [2 * p, 2 * p + 1]
                                   for p in range(n_pair)}
                    po = {}
                    for o4 in range(4):
                        for h in range(2):
                            po[(o4, h)] = ppool.tile(
                                [128, 512], F32, tag="ps",
                                name=f"po_{bs}_{oh}_{o4}_{h}")
                    obs = {o4: opool.tile([128, 1024], BF16, tag="o",
                                          name=f"ob_{bs}_{oh}_{o4}")
                           for o4 in range(4)}
                    for pair in range(n_pair):
                        sect = base_before[pair]
                        for ci in sect:
                            bp, plane = ci // 2, ci % 2
                            for o4 in range(4):
                                lhsT = wbt[bp][
                                    :, oh * 1024 + plane * 512
                                    + o4 * 128:
                                    oh * 1024 + plane * 512
                                    + (o4 + 1) * 128]
                                for h in range(2):
                                    nc.tensor.matmul(
                                        po[(o4, h)][:], lhsT,
                                        xts[ci // 2][
                                            :, plane,
                                            bs * F + h * 512:
                                            bs * F + (h + 1) * 512],
                                        start=(pair == 0
                                               and ci == sect[0]),
                                        stop=False)
                        wm = load_w8(bs, oh, pair)
                        # prefetch next cheby weights
                        if pair + 1 < n_pair:
                            load_w8(bs, oh, pair + 1)
                        elif oh + 1 < n_oh:
                            load_w8(bs, oh + 1, 0)
                        elif bs + 1 < n_bs:
                            load_w8(bs + 1, 0, 0)
                        wmv = wm[:].rearrange("p (d two o) -> p d two o",
                                              d=DEG, two=2)
                        t8p = t8s[(bs, pair)]
                        if first_pass:
                            # h-major + d-major: consume in exactly the
                            # order the (column-split) basis production
                            # delivers tiles
                            for h in range(2):
                                for d in range(1, DEG + 1):
                                    lhsT = wmv[:, d - 1, :, :]
                                    for o4 in range(4):
                                        nc.tensor.matmul(
                                            po[(o4, h)][:],
                                            lhsT[:, :,
                                                 o4 * 128:
                                                 (o4 + 1) * 128],
                                            t8p[:, d - 1, :,
                                                h * 512:(h + 1) * 512],
                                            start=False,
                                            stop=(pair == n_pair - 1
                                                  and d == DEG),
                                            perf_mode=DR)
                                if pair == n_pair - 1:
                                    for o4 in range(4):
                                        evict(bs, oh, o4, h,
                                              po[(o4, h)],
                                              on_scalar=((o4 + h) % 2
                                                         == 0),
                                              obt=obs[o4])
                        elif pair < n_pair - 1:
                            for o4 in range(4):
                                for d in range(1, DEG + 1):
                                    lhsT = wmv[:, d - 1, :,
                                               o4 * 128:(o4 + 1) * 128]
                                    for h in range(2):
                                        nc.tensor.matmul(
                                            po[(o4, h)][:], lhsT,
                                            t8p[:, d - 1, :,
                                                h * 512:(h + 1) * 512],
                                            start=False, stop=False,
                                            perf_mode=DR)
                        else:
                            # last pair: h-major per o4 so each
                            # single-bank PSUM tile stops (and evicts)
                            # as early as possible
                            for o4 in range(4):
                                for h in range(2):
                                    for d in range(1, DEG + 1):
                                        lhsT = wmv[:, d - 1, :,
                                                   o4 * 128:
                                                   (o4 + 1) * 128]
                                        nc.tensor.matmul(
                                            po[(o4, h)][:], lhsT,
                                            t8p[:, d - 1, :,
                                                h * 512:(h + 1) * 512],
                                            start=False,
                                            stop=(d == DEG),
                                            perf_mode=DR)
                                    evict(bs, oh, o4, h, po[(o4, h)],
                                          on_scalar=((o4 + h) % 2
                                                     == 0),
                                          obt=obs[o4],
                                          fin=(bs == n_bs - 1
                                               and oh == n_oh - 1
                                               and o4 == 3 and h == 1))
    nc.compile()
    return nc


def _prep_weights(cheby_coeffs: np.ndarray, base_weight: np.ndarray):
    C = np.asarray(cheby_coeffs, dtype=np.float32)
    BW = np.asarray(base_weight, dtype=np.float32)
    # cheby fp8 mega-tiles: [oh, pair, k, d(1..7), plane, o(512)].
    # Device basis tile d holds GAMMA[d-1]*T_d, so weight = C*SC/gamma.
    scl = (SC / GAMMA).astype(np.float32)
    Cs = C[:, :, 1:] * scl                             # [o, i, d]
    W8 = Cs.reshape(2, 512, 4, 2, 128, DEG)            # [oh,o,pair,pl,k,d]
    W8 = np.ascontiguousarray(W8.transpose(0, 2, 4, 5, 3, 1))
    w8 = W8.astype(mybir.dt.np(FP8)).reshape(2, 4, 128, DEG * 2 * 512)
    w8 = np.ascontiguousarray(w8)
    # base bf16: [bpair, k, (oh, plane, o)], prescaled by SC
    wbs = (BW.T * SC).reshape(4, 2, 128, 2, 512)       # [bp,pl,k,oh,o]
    wbh = wbs.transpose(0, 2, 3, 1, 4).reshape(4, 128, 2048)
    wbh = np.ascontiguousarray(wbh.astype(mybir.dt.np(BF16)))
    bias = C[:, :, 0].sum(axis=1)
    biasm = np.ascontiguousarray(bias.reshape(8, 128).T)
    return w8, wbh, biasm


_PROGRAM_CACHE = {}


def _make_in_maps(x, cheby_coeffs, base_weight):
    x = np.asarray(x, dtype=np.float32)
    b_core = x.shape[0] // N_CORES
    w8, wbh, biasm = _prep_weights(cheby_coeffs, base_weight)
    xTb = np.ascontiguousarray(x.T.astype(mybir.dt.np(BF16)))
    in_maps = []
    for c in range(N_CORES):
        in_maps.append({
            "xT": np.ascontiguousarray(
                xTb[:, c * b_core:(c + 1) * b_core]),
            "w8": w8,
            "wb": wbh,
            "biasm": biasm,
        })
    return in_maps


def kernel(x: np.ndarray, cheby_coeffs: np.ndarray,
           base_weight: np.ndarray) -> np.ndarray:
    x = np.asarray(x, dtype=np.float32)
    b_full = x.shape[0]
    assert b_full % N_CORES == 0
    b_core = b_full // N_CORES

    key = (b_core, N_CORES)
    if key not in _PROGRAM_CACHE:
        _PROGRAM_CACHE[key] = _build_program(b_core)
    nc = _PROGRAM_CACHE[key]

    in_maps = _make_in_maps(x, cheby_coeffs, base_weight)
    res = run_bass_kernel_spmd(nc, in_maps, core_ids=list(range(N_CORES)))
    out = np.empty((b_full, OUT_F), dtype=np.float32)
    for c in range(N_CORES):
        out[c * b_core:(c + 1) * b_core] = \
            res.results[c]["outT"].astype(np.float32).T
    return out
